# revision 1
# baseline (speedup 1.0000x reference)
"""2-layer GCN (GCNConv -> relu -> GCNConv -> log_softmax) on 8 NeuronCores.

Strategy (standard distributed GNN data parallel):
  - nodes partitioned into 8 contiguous shards; core c owns dst shard c
  - edges partitioned by dst owner; within a core, bucketed by
    (src-octant "group", dst-chunk) and sorted by dst
  - per-layer aggregation on device:
      table   = dis * (features)          [16 feature-partitions x 8 src-octant groups]
      gather  = gpsimd ap_gather (per-group index streams)
      segsum  = DVE segmented scan (mask resets at dst-run starts)
      extract = gpsimd ap_gather of run-end prefix values
      combine = PE matmul with a group-summing 0/1 matrix
  - self-loop term handled analytically (dis_d^2 * h), bias/relu/W2/log_softmax
    fused in the epilogue
  - halo exchange of the (tiny) transformed features between launches is done
    on the host (concat of per-core shard outputs)

All floating point math runs on device in fp32 (masks are exact 0/1 bf16).
Host only does integer graph preprocessing and data movement.
"""
import os
import sys

sys.path.insert(0, '/opt/trn_rl_repo')

import numpy as np
import ml_dtypes

from concourse import bass, bacc, mybir
import concourse.tile as tile
from concourse.masks import make_identity
from concourse.bass_utils import run_bass_kernel_spmd

F32 = mybir.dt.float32
BF16 = mybir.dt.bfloat16
I16 = mybir.dt.int16
I32 = mybir.dt.int32

NCORES = 8
NG = 8  # src-octant groups (16 partitions each)

# accumulated HW time of the launches of the most recent kernel() call
LAST_EXEC_NS = []

_cache = {}


def _cdiv(a, b):
    return (a + b - 1) // b


# ----------------------------------------------------------------- launch A
def _build_launch_a(SH_PAD, IN, HID):
    NTILE = SH_PAD // 128
    nc = bacc.Bacc("TRN2", target_bir_lowering=False, debug=False, num_devices=NCORES)
    xs_d = nc.dram_tensor("xs", [SH_PAD, IN], F32, kind="ExternalInput")
    deg_d = nc.dram_tensor("deg2d", [128, NTILE], I32, kind="ExternalInput")
    w1_d = nc.dram_tensor("w1", [IN, HID], F32, kind="ExternalInput")
    h1sT_d = nc.dram_tensor("h1sT", [HID, SH_PAD], F32, kind="ExternalOutput")
    dis_d = nc.dram_tensor("dis2d", [128, NTILE], F32, kind="ExternalOutput")

    with tile.TileContext(nc) as tc:
        with (
            tc.tile_pool(name="persist", bufs=1) as pp,
            tc.tile_pool(name="loop", bufs=6) as lp,
            tc.tile_pool(name="psum", bufs=4, space="PSUM") as psp,
            tc.tile_pool(name="psum2", bufs=3, space="PSUM") as psp2,
        ):
            ident = pp.tile([128, 128], F32)
            make_identity(nc, ident[:])
            w1 = pp.tile([IN, HID], F32)
            nc.sync.dma_start(out=w1[:], in_=w1_d[:])
            degt = pp.tile([128, NTILE], I32)
            nc.sync.dma_start(out=degt[:], in_=deg_d[:])
            degf = pp.tile([128, NTILE], F32)
            nc.vector.tensor_copy(out=degf[:], in_=degt[:])
            dis = pp.tile([128, NTILE], F32)
            nc.vector.reciprocal(out=dis[:], in_=degf[:])
            nc.scalar.activation(dis[:], dis[:], mybir.ActivationFunctionType.Sqrt)
            nc.sync.dma_start(out=dis_d[:], in_=dis[:])

            h1sT = pp.tile([HID, SH_PAD], F32)
            for t in range(NTILE):
                xt = lp.tile([128, IN], F32, tag="xt")
                nc.sync.dma_start(out=xt[:], in_=xs_d[t * 128:(t + 1) * 128, :])
                nc.vector.tensor_tensor(
                    out=xt[:], in0=xt[:],
                    in1=dis[:, t:t + 1].to_broadcast([128, IN]),
                    op=mybir.AluOpType.mult)
                pT = psp.tile([128, 128], F32, tag="pT")
                nc.tensor.transpose(out=pT[:, :IN], in_=xt[:], identity=ident[:])
                xT = lp.tile([IN, 128], F32, tag="xT")
                nc.vector.tensor_copy(out=xT[:], in_=pT[:IN, :])
                ph = psp2.tile([HID, 128], F32, tag="ph")
                nc.tensor.matmul(out=ph[:], lhsT=w1[:], rhs=xT[:],
                                 start=True, stop=True)
                nc.vector.tensor_copy(out=h1sT[:, t * 128:(t + 1) * 128], in_=ph[:])
            nc.sync.dma_start(out=h1sT_d[:], in_=h1sT[:])
    nc.compile()
    return nc


# --------------------------------------------------------------- launch B/C
def _build_launch_agg(W, C, NCHUNK, DST_CH, DST_PAD, HID, OUT, layer):
    """layer=1: combine->+self->*dis->+b1->relu->W2->*dis -> z [2, DST_PAD]
    layer=2: combine->+self->*dis->+b2 -> log_softmax -> out [2, 128, SMR]"""
    F = HID if layer == 1 else OUT
    SMR = DST_PAD // 128
    nc = bacc.Bacc("TRN2", target_bir_lowering=False, debug=False, num_devices=NCORES)
    table_d = nc.dram_tensor("table", [128, W], F32, kind="ExternalInput")
    idx_d = nc.dram_tensor("idx", [128, NCHUNK * (C // 16)], I16, kind="ExternalInput")
    mask_d = nc.dram_tensor("mask", [128, NCHUNK * C], BF16, kind="ExternalInput")
    ext_d = nc.dram_tensor("ext", [128, NCHUNK * (DST_CH // 16)], I16, kind="ExternalInput")
    disr_d = nc.dram_tensor("disrep", [F, DST_PAD], F32, kind="ExternalInput")
    self_d = nc.dram_tensor("selfv", [F, DST_PAD], F32, kind="ExternalInput")
    bias_d = nc.dram_tensor("bias", [F, 1], F32, kind="ExternalInput")
    g_d = nc.dram_tensor("gmat", [128, F], F32, kind="ExternalInput")
    if layer == 1:
        w2_d = nc.dram_tensor("w2", [HID, OUT], F32, kind="ExternalInput")
        out_d = nc.dram_tensor("z", [OUT, DST_PAD], F32, kind="ExternalOutput")
    else:
        out_d = nc.dram_tensor("o", [OUT, 128, SMR], F32, kind="ExternalOutput")

    NS = DST_CH // 512  # 512-col epilogue slices per chunk

    with tile.TileContext(nc) as tc:
        with (
            tc.tile_pool(name="persist", bufs=1) as pp,
            tc.tile_pool(name="loop", bufs=2) as lp,
            tc.tile_pool(name="big", bufs=2) as bigp,
            tc.tile_pool(name="ep", bufs=2) as ep,
            tc.tile_pool(name="epin", bufs=1) as epin,
            tc.tile_pool(name="psA", bufs=2, space="PSUM") as psA,
            tc.tile_pool(name="psB", bufs=2, space="PSUM") as psB,
            tc.tile_pool(name="dram", bufs=1, space="DRAM") as dp,
        ):
            table = pp.tile([128, W], F32)
            nc.sync.dma_start(out=table[:], in_=table_d[:])
            gmat = pp.tile([128, F], F32)
            nc.sync.dma_start(out=gmat[:], in_=g_d[:])
            bias = pp.tile([F, 1], F32)
            nc.sync.dma_start(out=bias[:], in_=bias_d[:])
            if layer == 1:
                w2 = pp.tile([HID, OUT], F32)
                nc.sync.dma_start(out=w2[:], in_=w2_d[:])
            if layer == 2:
                z2 = dp.tile([OUT, DST_PAD], F32)

            for k in range(NCHUNK):
                idx_t = lp.tile([128, C // 16], I16, tag="idx")
                nc.sync.dma_start(out=idx_t[:], in_=idx_d[:, k * (C // 16):(k + 1) * (C // 16)])
                mask_t = epin.tile([128, C], BF16, tag="mask")
                nc.sync.dma_start(out=mask_t[:], in_=mask_d[:, k * C:(k + 1) * C])
                ext_t = lp.tile([128, DST_CH // 16], I16, tag="ext")
                nc.sync.dma_start(out=ext_t[:], in_=ext_d[:, k * (DST_CH // 16):(k + 1) * (DST_CH // 16)])
                self_t = epin.tile([F, DST_CH], F32, tag="selfv")
                nc.sync.dma_start(out=self_t[:], in_=self_d[:, k * DST_CH:(k + 1) * DST_CH])
                disr_t = epin.tile([F, DST_CH], F32, tag="disr")
                nc.sync.dma_start(out=disr_t[:], in_=disr_d[:, k * DST_CH:(k + 1) * DST_CH])

                msg = bigp.tile([128, C], F32, tag="msg")
                nc.gpsimd.ap_gather(
                    out_ap=msg[:], in_ap=table[:], idxs_ap=idx_t[:],
                    channels=128, num_elems=W, d=1, num_idxs=C)
                csum = bigp.tile([128, C + 16], F32, tag="csum")
                nc.vector.memset(csum[:, 0:1], 0.0)
                nc.vector.tensor_tensor_scan(
                    out=csum[:, 1:C + 1], data0=mask_t[:], data1=msg[:],
                    initial=0.0, op0=mybir.AluOpType.mult, op1=mybir.AluOpType.add)
                extv = lp.tile([128, DST_CH], F32, tag="extv")
                nc.gpsimd.ap_gather(
                    out_ap=extv[:], in_ap=csum[:, 0:C + 1], idxs_ap=ext_t[:],
                    channels=128, num_elems=C + 1, d=1, num_idxs=DST_CH)

                for s in range(NS):
                    sl = slice(s * 512, (s + 1) * 512)
                    ps = psA.tile([F, 512], F32, tag="ps")
                    nc.tensor.matmul(out=ps[:], lhsT=gmat[:], rhs=extv[:, sl],
                                     start=True, stop=True)
                    a1 = ep.tile([F, 512], F32, tag="a1")
                    nc.vector.tensor_tensor(out=a1[:], in0=ps[:], in1=self_t[:, sl],
                                            op=mybir.AluOpType.add)
                    nc.vector.tensor_tensor(out=a1[:], in0=a1[:], in1=disr_t[:, sl],
                                            op=mybir.AluOpType.mult)
                    nc.vector.tensor_tensor(out=a1[:], in0=a1[:],
                                            in1=bias[:].to_broadcast([F, 512]),
                                            op=mybir.AluOpType.add)
                    if layer == 1:
                        nc.vector.tensor_scalar_max(a1[:], a1[:], 0.0)
                        ps2 = psB.tile([OUT, 512], F32, tag="ps2")
                        nc.tensor.matmul(out=ps2[:], lhsT=w2[:], rhs=a1[:],
                                         start=True, stop=True)
                        zt = ep.tile([OUT, 512], F32, tag="zt")
                        nc.vector.tensor_tensor(out=zt[:], in0=ps2[:],
                                                in1=disr_t[:OUT, sl],
                                                op=mybir.AluOpType.mult)
                        nc.sync.dma_start(
                            out=out_d[:, k * DST_CH + s * 512:k * DST_CH + (s + 1) * 512],
                            in_=zt[:])
                    else:
                        nc.sync.dma_start(
                            out=z2[:, k * DST_CH + s * 512:k * DST_CH + (s + 1) * 512],
                            in_=a1[:])

            if layer == 2:
                # log_softmax over the 2 classes, done in [128, SMR] layout
                z0 = pp.tile([128, SMR], F32)
                z1 = pp.tile([128, SMR], F32)
                nc.sync.dma_start(out=z0[:], in_=z2[0:1, :].rearrange('o (p f) -> (o p) f', p=128))
                nc.sync.dma_start(out=z1[:], in_=z2[1:2, :].rearrange('o (p f) -> (o p) f', p=128))
                m = pp.tile([128, SMR], F32)
                nc.vector.tensor_tensor(out=m[:], in0=z0[:], in1=z1[:], op=mybir.AluOpType.max)
                d0 = pp.tile([128, SMR], F32)
                d1 = pp.tile([128, SMR], F32)
                nc.vector.tensor_tensor(out=d0[:], in0=z0[:], in1=m[:], op=mybir.AluOpType.subtract)
                nc.vector.tensor_tensor(out=d1[:], in0=z1[:], in1=m[:], op=mybir.AluOpType.subtract)
                e0 = pp.tile([128, SMR], F32)
                e1 = pp.tile([128, SMR], F32)
                nc.scalar.activation(e0[:], d0[:], mybir.ActivationFunctionType.Exp)
                nc.scalar.activation(e1[:], d1[:], mybir.ActivationFunctionType.Exp)
                nc.vector.tensor_tensor(out=e0[:], in0=e0[:], in1=e1[:], op=mybir.AluOpType.add)
                ls = pp.tile([128, SMR], F32)
                nc.scalar.activation(ls[:], e0[:], mybir.ActivationFunctionType.Ln)
                nc.vector.tensor_tensor(out=d0[:], in0=d0[:], in1=ls[:], op=mybir.AluOpType.subtract)
                nc.vector.tensor_tensor(out=d1[:], in0=d1[:], in1=ls[:], op=mybir.AluOpType.subtract)
                nc.sync.dma_start(out=out_d[0], in_=d0[:])
                nc.sync.dma_start(out=out_d[1], in_=d1[:])
    nc.compile()
    return nc


# ------------------------------------------------------------- preprocessing
def _preprocess(edge_index, N, SH, DST_CH, NCHUNK):
    src = np.asarray(edge_index[0]).astype(np.int64)
    dst = np.asarray(edge_index[1]).astype(np.int64)
    E = src.shape[0]
    deg = (np.bincount(dst, minlength=N) + 1).astype(np.int32)

    core = (dst // SH).astype(np.int64)
    grp = (src // SH).astype(np.int64)
    order = np.lexsort((dst, grp, core))
    s_s = src[order]
    d_s = dst[order]
    c_s = core[order]
    g_s = grp[order]
    chunk = ((d_s % SH) // DST_CH).astype(np.int64)
    bucket = (c_s * NG + g_s) * NCHUNK + chunk
    nb = NCORES * NG * NCHUNK
    counts = np.bincount(bucket, minlength=nb)
    C = int(_cdiv(max(int(counts.max()), 16), 16) * 16)
    assert C + 1 < 32768, C  # extraction positions must fit int16
    offs = np.zeros(nb + 1, np.int64)
    np.cumsum(counts, out=offs[1:])
    pos = np.arange(E, dtype=np.int64) - offs[bucket]

    first = np.ones(E, bool)
    first[1:] = (d_s[1:] != d_s[:-1]) | (bucket[1:] != bucket[:-1])
    last = np.ones(E, bool)
    last[:-1] = first[1:]

    idx_all = np.full((NCORES, 128, NCHUNK * (C // 16)), SH, np.int16)
    p_part = (16 * g_s + pos % 16).astype(np.int64)
    p_col = (chunk * (C // 16) + pos // 16).astype(np.int64)
    idx_all[c_s, p_part, p_col] = (s_s - g_s * SH).astype(np.int16)

    maskg = np.ones((NCORES, NG, NCHUNK * C), np.float32)
    maskg[c_s[first], g_s[first], (chunk[first] * C + pos[first])] = 0.0
    mask_all = np.repeat(maskg, 16, axis=1).astype(ml_dtypes.bfloat16)

    ext_all = np.zeros((NCORES, 128, NCHUNK * (DST_CH // 16)), np.int16)
    le = np.nonzero(last)[0]
    slot = (d_s[le] % SH) % DST_CH
    ext_all[c_s[le], (16 * g_s[le] + slot % 16), (chunk[le] * (DST_CH // 16) + slot // 16)] = \
        (pos[le] + 1).astype(np.int16)

    return deg, C, idx_all, mask_all, ext_all


# ---------------------------------------------------------------------- main
def kernel(x, edge_index, W1, b1, W2, b2):
    global LAST_EXEC_NS
    LAST_EXEC_NS = []
    x = np.asarray(x, np.float32)
    W1 = np.asarray(W1, np.float32)
    b1 = np.asarray(b1, np.float32)
    W2 = np.asarray(W2, np.float32)
    b2 = np.asarray(b2, np.float32)
    N, IN = x.shape
    HID = W1.shape[1]
    OUT = W2.shape[1]
    assert N % NCORES == 0
    SH = N // NCORES
    assert SH + 1 < 32768, SH  # local node ids + zero sentinel must fit int16
    SH_PAD = _cdiv(SH, 128) * 128
    W = SH_PAD  # gather-table columns; col SH.. are zero (pad sentinel = SH)
    DST_CH = 1024 if SH >= 1024 else max(512, _cdiv(SH, 512) * 512)
    NCHUNK = _cdiv(SH, DST_CH)
    DST_PAD = NCHUNK * DST_CH
    assert DST_PAD % 128 == 0
    trace = bool(os.environ.get("BASS_TRACE"))

    deg, C, idx_all, mask_all, ext_all = _preprocess(edge_index, N, SH, DST_CH, NCHUNK)

    # ---- launch A: per-shard h1sT = dis * (x @ W1)^T
    key_a = ("A", SH_PAD, IN, HID)
    if key_a not in _cache:
        _cache[key_a] = _build_launch_a(SH_PAD, IN, HID)
    nc_a = _cache[key_a]
    NTILE = SH_PAD // 128
    in_maps = []
    for c in range(NCORES):
        xs = np.zeros((SH_PAD, IN), np.float32)
        xs[:SH] = x[c * SH:(c + 1) * SH]
        dsh = np.ones(SH_PAD, np.int32)
        dsh[:SH] = deg[c * SH:(c + 1) * SH]
        in_maps.append({"xs": xs, "deg2d": np.ascontiguousarray(dsh.reshape(NTILE, 128).T),
                        "w1": W1})
    res_a = run_bass_kernel_spmd(nc_a, in_maps, list(range(NCORES)), trace=trace)
    LAST_EXEC_NS.append(res_a.exec_time_ns)
    h1sT = [res_a.results[c]["h1sT"] for c in range(NCORES)]          # [HID, SH_PAD]
    disf = [np.ascontiguousarray(res_a.results[c]["dis2d"].T).reshape(-1)
            for c in range(NCORES)]                                    # [SH_PAD]

    # ---- assemble shared/derived host arrays
    tableB = np.zeros((128, W), np.float32)
    for j in range(NG):
        tableB[16 * j:16 * j + HID, :] = h1sT[j]
        tableB[16 * j:16 * j + HID, SH:] = 0.0
    g1 = np.zeros((128, HID), np.float32)
    for j in range(NG):
        g1[16 * j + np.arange(HID), np.arange(HID)] = 1.0
    g2 = np.zeros((128, OUT), np.float32)
    for j in range(NG):
        g2[16 * j + np.arange(OUT), np.arange(OUT)] = 1.0

    def disrep(c, F):
        d = np.ones(DST_PAD, np.float32)
        d[:SH] = disf[c][:SH]
        return np.tile(d[None, :], (F, 1))

    # ---- launch B: layer-1 aggregation + relu + W2 -> z shards
    key_b = ("B", W, C, NCHUNK, DST_CH, DST_PAD, HID, OUT, 1)
    if key_b not in _cache:
        _cache[key_b] = _build_launch_agg(W, C, NCHUNK, DST_CH, DST_PAD, HID, OUT, 1)
    nc_b = _cache[key_b]
    in_maps = []
    for c in range(NCORES):
        selfv = np.zeros((HID, DST_PAD), np.float32)
        selfv[:, :SH] = h1sT[c][:, :SH]
        in_maps.append({
            "table": tableB, "idx": idx_all[c], "mask": mask_all[c], "ext": ext_all[c],
            "disrep": disrep(c, HID), "selfv": selfv,
            "bias": b1.reshape(HID, 1), "gmat": g1, "w2": W2,
        })
    res_b = run_bass_kernel_spmd(nc_b, in_maps, list(range(NCORES)), trace=trace)
    LAST_EXEC_NS.append(res_b.exec_time_ns)
    zs = [res_b.results[c]["z"] for c in range(NCORES)]               # [OUT, DST_PAD]

    # ---- launch C: layer-2 aggregation + bias + log_softmax
    tableC = np.zeros((128, W), np.float32)
    for j in range(NG):
        tableC[16 * j:16 * j + OUT, :SH] = zs[j][:, :SH]
    key_c = ("C", W, C, NCHUNK, DST_CH, DST_PAD, HID, OUT, 2)
    if key_c not in _cache:
        _cache[key_c] = _build_launch_agg(W, C, NCHUNK, DST_CH, DST_PAD, HID, OUT, 2)
    nc_c = _cache[key_c]
    in_maps = []
    for c in range(NCORES):
        selfv = np.zeros((OUT, DST_PAD), np.float32)
        selfv[:, :SH] = zs[c][:, :SH]
        in_maps.append({
            "table": tableC, "idx": idx_all[c], "mask": mask_all[c], "ext": ext_all[c],
            "disrep": disrep(c, OUT), "selfv": selfv,
            "bias": b2.reshape(OUT, 1), "gmat": g2,
        })
    res_c = run_bass_kernel_spmd(nc_c, in_maps, list(range(NCORES)), trace=trace)
    LAST_EXEC_NS.append(res_c.exec_time_ns)

    out = np.empty((N, OUT), np.float32)
    for c in range(NCORES):
        o = res_c.results[c]["o"].reshape(OUT, DST_PAD)
        out[c * SH:(c + 1) * SH] = o[:, :SH].T
    return out



# revision 2
# speedup vs baseline: 1.0161x; 1.0161x over previous
"""2-layer GCN (GCNConv -> relu -> GCNConv -> log_softmax) on 8 NeuronCores.

Strategy (distributed GNN data parallel, dst-sharded):
  - nodes partitioned into 8 contiguous dst-shards; core c owns shard c
  - per-shard neighbor lists are degree-sorted and K-padded into uniform
    rank blocks of 128 dsts (shared K profile across cores => one NEFF)
  - launch A (per core): dis = rsqrt(deg); h1T = W1^T @ x^T scaled by
    dis[s], emitted as bf16 node rows (the layer-1 message table)
  - host exchanges halo rows: the per-edge message streams for each core
    are assembled by pure byte movement (np.take of device-produced bf16
    rows, zero-padding via a sentinel row) - no host float arithmetic
  - launch B (per core): stream msg1, segment-reduce over K slots on DVE,
    epilogue: *dis[d] +b1, relu, @W2 (via broadcast-multiply + reduce),
    *dis[d]  -> layer-2 message rows (bf16)
  - host exchanges halo rows again (msg2 assembly)
  - launch C (per core): stream msg2, segment-reduce, *dis[d] +b2,
    log_softmax -> output rows

All floating point math runs on device in fp32 (message streams are bf16).
Host does integer graph preprocessing and data movement only.
"""
import os
import sys

sys.path.insert(0, '/opt/trn_rl_repo')

import numpy as np
import ml_dtypes

from concourse import bass, bacc, mybir
import concourse.tile as tile
from concourse.bass_utils import run_bass_kernel_spmd

F32 = mybir.dt.float32
BF16 = mybir.dt.bfloat16
I32 = mybir.dt.int32

NCORES = 8

LAST_EXEC_NS = []

_cache = {}


# ----------------------------------------------------------------- launch A
def _build_launch_a(SHP, IN, HID):
    """Per core: dis2d = rsqrt(deg2d);  h1T[:, n] = dis[n] * (W1^T x[n])
    in bf16. Node id n = p*R + t (natural shard order; deg2d is the
    p-major reshape of the shard's deg)."""
    R = SHP // 128
    NSL = SHP // 512  # 512-column matmul slices
    nc = bacc.Bacc("TRN2", target_bir_lowering=False, debug=False, num_devices=NCORES)
    xT_d = nc.dram_tensor("xT", [IN, SHP], F32, kind="ExternalInput")
    deg_d = nc.dram_tensor("deg2d", [128, R], I32, kind="ExternalInput")
    w1_d = nc.dram_tensor("w1", [IN, HID], F32, kind="ExternalInput")
    h1T_d = nc.dram_tensor("h1T", [HID, SHP], BF16, kind="ExternalOutput")
    dis_d = nc.dram_tensor("dis2d", [128, R], F32, kind="ExternalOutput")

    with tile.TileContext(nc) as tc:
        with (
            tc.tile_pool(name="persist", bufs=1) as pp,
            tc.tile_pool(name="loop", bufs=3) as lp,
            tc.tile_pool(name="psum", bufs=4, space="PSUM") as psp,
            tc.tile_pool(name="dram", bufs=1, space="DRAM") as dp,
        ):
            w1 = pp.tile([IN, HID], F32)
            nc.sync.dma_start(out=w1[:], in_=w1_d[:])
            degt = pp.tile([128, R], I32)
            nc.sync.dma_start(out=degt[:], in_=deg_d[:])
            degf = pp.tile([128, R], F32)
            nc.vector.tensor_copy(out=degf[:], in_=degt[:])
            dis = pp.tile([128, R], F32)
            nc.vector.reciprocal(out=dis[:], in_=degf[:])
            nc.scalar.activation(dis[:], dis[:], mybir.ActivationFunctionType.Sqrt)
            nc.sync.dma_start(out=dis_d[:], in_=dis[:])
            # roundtrip: [128, R] p-major -> DRAM [SH] -> replicated [HID, SH]
            disl = dp.tile([1, SHP], F32)
            nc.sync.dma_start(out=disl[:], in_=dis[:])
            dis16 = pp.tile([HID, SHP], F32)
            for h in range(HID):
                nc.sync.dma_start(out=dis16[h:h + 1, :], in_=disl[:])

            h1T = pp.tile([HID, SHP], BF16)
            for s in range(NSL):
                sl = slice(s * 512, (s + 1) * 512)
                xt = lp.tile([IN, 512], F32, tag="xt")
                nc.sync.dma_start(out=xt[:], in_=xT_d[:, sl])
                ph = psp.tile([HID, 512], F32, tag="ph")
                nc.tensor.matmul(out=ph[:], lhsT=w1[:], rhs=xt[:],
                                 start=True, stop=True)
                nc.vector.tensor_tensor(out=h1T[:, sl], in0=ph[:],
                                        in1=dis16[:, sl], op=mybir.AluOpType.mult)
            nc.sync.dma_start(out=h1T_d[:], in_=h1T[:])
    nc.compile()
    return nc


# --------------------------------------------------------------- launch B
def _build_launch_b(SHP, HID, OUT, spans, chunks, NSLOT):
    """Stream msg1 [128, NSLOT*HID] bf16, reduce each span's K slots,
    epilogue -> z rows (bf16) for layer 2."""
    R = SHP // 128
    nc = bacc.Bacc("TRN2", target_bir_lowering=False, debug=False, num_devices=NCORES)
    msg_d = nc.dram_tensor("msg1", [128, NSLOT * HID], BF16, kind="ExternalInput")
    disrep_d = nc.dram_tensor("disrep", [128, R * HID], F32, kind="ExternalInput")
    b1rep_d = nc.dram_tensor("b1rep", [128, R * HID], F32, kind="ExternalInput")
    w2rep0_d = nc.dram_tensor("w2rep0", [128, R * HID], F32, kind="ExternalInput")
    w2rep1_d = nc.dram_tensor("w2rep1", [128, R * HID], F32, kind="ExternalInput")
    disP_d = nc.dram_tensor("disP", [128, R], F32, kind="ExternalInput")
    z0_d = nc.dram_tensor("z0", [128, R], BF16, kind="ExternalOutput")
    z1_d = nc.dram_tensor("z1", [128, R], BF16, kind="ExternalOutput")

    with tile.TileContext(nc) as tc:
        with (
            tc.tile_pool(name="persist", bufs=1) as pp,
            tc.tile_pool(name="msg", bufs=2) as mp,
        ):
            disrep = pp.tile([128, R * HID], F32)
            nc.sync.dma_start(out=disrep[:], in_=disrep_d[:])
            b1rep = pp.tile([128, R * HID], F32)
            nc.sync.dma_start(out=b1rep[:], in_=b1rep_d[:])
            agg = pp.tile([128, R * HID], F32)
            for (s0, s1, r0a, spl) in chunks:
                nslots = s1 - s0
                m = mp.tile([128, nslots * HID], BF16, tag="m")
                nc.sync.dma_start(out=m[:], in_=msg_d[:, s0 * HID:s1 * HID])
                for (K, r0, r1, soff) in spl:
                    nr = r1 - r0
                    view = m[:, (soff - s0) * HID:(soff - s0 + nr * K) * HID] \
                        .rearrange('p (nr f k) -> p nr f k', f=HID, k=K)
                    nc.vector.tensor_reduce(
                        out=agg[:, r0 * HID:r1 * HID], in_=view,
                        axis=mybir.AxisListType.X, op=mybir.AluOpType.add)
            # epilogue: h = relu(agg*disrep + b1)
            nc.vector.tensor_tensor(out=agg[:], in0=agg[:], in1=disrep[:],
                                    op=mybir.AluOpType.mult)
            nc.vector.tensor_tensor(out=agg[:], in0=agg[:], in1=b1rep[:],
                                    op=mybir.AluOpType.add)
            nc.vector.tensor_scalar_max(agg[:], agg[:], 0.0)
            # z_o = (sum_f h*w2rep_o) * disP  -> bf16
            disP = pp.tile([128, R], F32)
            nc.sync.dma_start(out=disP[:], in_=disP_d[:])
            tmp = pp.tile([128, R * HID], F32)
            for o, (w_d, z_d) in enumerate(((w2rep0_d, z0_d), (w2rep1_d, z1_d))):
                w2rep = mp.tile([128, R * HID], F32, tag="w2rep")
                nc.sync.dma_start(out=w2rep[:], in_=w_d[:])
                nc.vector.tensor_tensor(out=tmp[:], in0=agg[:], in1=w2rep[:],
                                        op=mybir.AluOpType.mult)
                z = pp.tile([128, R], F32)
                nc.vector.tensor_reduce(
                    out=z[:], in_=tmp[:].rearrange('p (r f) -> p r f', f=HID),
                    axis=mybir.AxisListType.X, op=mybir.AluOpType.add)
                nc.vector.tensor_tensor(out=z[:], in0=z[:], in1=disP[:],
                                        op=mybir.AluOpType.mult)
                zb = pp.tile([128, R], BF16)
                nc.vector.tensor_copy(out=zb[:], in_=z[:])
                nc.sync.dma_start(out=z_d[:], in_=zb[:])
    nc.compile()
    return nc


# --------------------------------------------------------------- launch C
def _build_launch_c(SHP, OUT, spans, NSLOT):
    """Stream msg2 [128, NSLOT*OUT] bf16, reduce, *dis +b2, log_softmax."""
    R = SHP // 128
    nc = bacc.Bacc("TRN2", target_bir_lowering=False, debug=False, num_devices=NCORES)
    msg_d = nc.dram_tensor("msg2", [128, NSLOT * OUT], BF16, kind="ExternalInput")
    disrep_d = nc.dram_tensor("disrep2", [128, R * OUT], F32, kind="ExternalInput")
    b2rep_d = nc.dram_tensor("b2rep", [128, R * OUT], F32, kind="ExternalInput")
    o0_d = nc.dram_tensor("o0", [128, R], F32, kind="ExternalOutput")
    o1_d = nc.dram_tensor("o1", [128, R], F32, kind="ExternalOutput")

    with tile.TileContext(nc) as tc:
        with (
            tc.tile_pool(name="persist", bufs=1) as pp,
            tc.tile_pool(name="msg", bufs=2) as mp,
        ):
            agg = pp.tile([128, R * OUT], F32)
            half = NSLOT // 2
            # split spans at ~half for double buffering
            bnds = [0]
            acc = 0
            spl_chunks = [[]]
            for (K, r0, r1, soff) in spans:
                if acc >= half and len(spl_chunks) == 1:
                    spl_chunks.append([])
                    bnds.append(acc)
                spl_chunks[-1].append((K, r0, r1, soff))
                acc = soff + (r1 - r0) * K
            bnds.append(NSLOT)
            for ci, spl in enumerate(spl_chunks):
                s0, s1 = bnds[ci], bnds[ci + 1]
                if s1 <= s0:
                    continue
                m = mp.tile([128, (s1 - s0) * OUT], BF16, tag="m")
                nc.sync.dma_start(out=m[:], in_=msg_d[:, s0 * OUT:s1 * OUT])
                for (K, r0, r1, soff) in spl:
                    nr = r1 - r0
                    view = m[:, (soff - s0) * OUT:(soff - s0 + nr * K) * OUT] \
                        .rearrange('p (nr f k) -> p nr f k', f=OUT, k=K)
                    nc.vector.tensor_reduce(
                        out=agg[:, r0 * OUT:r1 * OUT], in_=view,
                        axis=mybir.AxisListType.X, op=mybir.AluOpType.add)
            disrep = pp.tile([128, R * OUT], F32)
            nc.sync.dma_start(out=disrep[:], in_=disrep_d[:])
            b2rep = pp.tile([128, R * OUT], F32)
            nc.sync.dma_start(out=b2rep[:], in_=b2rep_d[:])
            nc.vector.tensor_tensor(out=agg[:], in0=agg[:], in1=disrep[:],
                                    op=mybir.AluOpType.mult)
            nc.vector.tensor_tensor(out=agg[:], in0=agg[:], in1=b2rep[:],
                                    op=mybir.AluOpType.add)
            # log_softmax over the 2 classes
            a3 = agg[:].rearrange('p (r o) -> p r o', o=OUT)
            z0v, z1v = a3[:, :, 0:1], a3[:, :, 1:2]
            mx = pp.tile([128, R], F32)
            nc.vector.tensor_tensor(out=mx[:], in0=z0v, in1=z1v, op=mybir.AluOpType.max)
            d0 = pp.tile([128, R], F32)
            d1 = pp.tile([128, R], F32)
            nc.vector.tensor_tensor(out=d0[:], in0=z0v, in1=mx[:], op=mybir.AluOpType.subtract)
            nc.vector.tensor_tensor(out=d1[:], in0=z1v, in1=mx[:], op=mybir.AluOpType.subtract)
            e0 = pp.tile([128, R], F32)
            e1 = pp.tile([128, R], F32)
            nc.scalar.activation(e0[:], d0[:], mybir.ActivationFunctionType.Exp)
            nc.scalar.activation(e1[:], d1[:], mybir.ActivationFunctionType.Exp)
            nc.vector.tensor_tensor(out=e0[:], in0=e0[:], in1=e1[:], op=mybir.AluOpType.add)
            ls = pp.tile([128, R], F32)
            nc.scalar.activation(ls[:], e0[:], mybir.ActivationFunctionType.Ln)
            nc.vector.tensor_tensor(out=d0[:], in0=d0[:], in1=ls[:], op=mybir.AluOpType.subtract)
            nc.vector.tensor_tensor(out=d1[:], in0=d1[:], in1=ls[:], op=mybir.AluOpType.subtract)
            nc.sync.dma_start(out=o0_d[:], in_=d0[:])
            nc.sync.dma_start(out=o1_d[:], in_=d1[:])
    nc.compile()
    return nc


# ------------------------------------------------------------- preprocessing
def _preprocess(edge_index, N, SH, SHP):
    """Degree-sorted K-padded per-core layouts (shared K profile).
    Shards padded from SH to SHP dsts (padded dsts: one sentinel slot)."""
    src = np.asarray(edge_index[0]).astype(np.int64)
    dst = np.asarray(edge_index[1]).astype(np.int64)
    deg = (np.bincount(dst, minlength=N) + 1).astype(np.int64)  # incl. self
    R = SHP // 128

    perms = []
    csrs = []   # (order-sorted srcs per core, start offsets per local dst)
    degps = []
    for c in range(NCORES):
        lo, hi = c * SH, (c + 1) * SH
        degp = np.ones(SHP, np.int64)
        degp[:SH] = deg[lo:hi]
        degps.append(degp)
        perm = np.argsort(-degp, kind='stable')   # local ids, degree desc
        perms.append(perm)
        sel = (dst >= lo) & (dst < hi)
        ds = dst[sel] - lo
        ss = src[sel]
        order = np.argsort(ds, kind='stable')
        ss = ss[order]
        counts = np.bincount(ds, minlength=SHP)
        starts = np.zeros(SHP + 1, np.int64)
        np.cumsum(counts, out=starts[1:])
        csrs.append((ss, starts))

    # shared K profile: K[r] = max over cores of the rank's max degree
    K = np.zeros(R, np.int64)
    for c in range(NCORES):
        sd = degps[c][perms[c]]
        K = np.maximum(K, sd[0::128])
    K = np.maximum(K, 1)

    # spans of equal K: (K, r0, r1, slot_offset)
    spans = []
    r0 = 0
    soff = 0
    for r in range(1, R + 1):
        if r == R or K[r] != K[r0]:
            spans.append((int(K[r0]), r0, r, soff))
            soff += (r - r0) * int(K[r0])
            r0 = r
    NSLOT = int(K.sum())

    # srcs_pad per core: [128, NSLOT] int32 global ids; sentinel = N
    srcs_all = np.empty((NCORES, 128, NSLOT), np.int32)
    for c in range(NCORES):
        perm = perms[c]
        P2 = perm.reshape(R, 128)                  # P2[r, p] = local dst id
        ss, starts = csrs[c]
        ss_ext = np.concatenate([ss, [np.int64(N)]])
        for (Kr, r0, r1, soff) in spans:
            for r in range(r0, r1):
                dl = P2[r]                          # [128] local ids
                dg = np.where(dl < SH, dl + c * SH, N)  # padded -> sentinel
                blk = np.full((128, Kr), N, np.int64)
                blk[:, 0] = dg                      # self-loop
                if Kr > 1:
                    lens = starts[dl + 1] - starts[dl]
                    ti = starts[dl][:, None] + np.arange(Kr - 1)[None, :]
                    valid = np.arange(Kr - 1)[None, :] < lens[:, None]
                    ti = np.where(valid, ti, len(ss))
                    blk[:, 1:] = ss_ext[ti]
                off = soff + (r - r0) * Kr
                srcs_all[c, :, off:off + Kr] = blk.astype(np.int32)

    return deg, degps, perms, K, spans, NSLOT, srcs_all


def _chunk_spans(spans, HID, budget_slots):
    """Split the span list into DMA chunks of <= budget_slots slots;
    large spans are split by ranks. Returns list of
    (s0, s1, r0, [(K, r0, r1, soff), ...])."""
    # first split big spans by rank
    fine = []
    for (K, r0, r1, soff) in spans:
        nr = r1 - r0
        max_nr = max(1, budget_slots // K)
        rr = r0
        so = soff
        while rr < r1:
            take = min(max_nr, r1 - rr)
            fine.append((K, rr, rr + take, so))
            so += take * K
            rr += take
    chunks = []
    cur = []
    s0 = 0
    acc = 0
    for sp in fine:
        K, r0, r1, soff = sp
        sz = (r1 - r0) * K
        if cur and acc + sz > budget_slots:
            chunks.append((s0, s0 + acc, cur[0][1], cur))
            s0 += acc
            cur = []
            acc = 0
        cur.append(sp)
        acc += sz
    if cur:
        chunks.append((s0, s0 + acc, cur[0][1], cur))
    return chunks


def _assemble_msg(table_ext, srcs, spans, F):
    """table_ext [N+1, F] bf16, srcs [128, NSLOT] int32 ->
    msg [128, NSLOT*F] bf16 with per-span layout (p, nr, F, K)."""
    NSLOT = srcs.shape[1]
    msg = np.empty((128, NSLOT * F), dtype=ml_dtypes.bfloat16)
    for (K, r0, r1, soff) in spans:
        nr = r1 - r0
        blk = table_ext[srcs[:, soff:soff + nr * K].reshape(128, nr, K)]
        # [128, nr, K, F] -> [128, nr, F, K]
        blk = np.ascontiguousarray(blk.transpose(0, 1, 3, 2))
        msg[:, soff * F:(soff + nr * K) * F] = blk.reshape(128, nr * K * F)
    return msg


# ---------------------------------------------------------------------- main
def kernel(x, edge_index, W1, b1, W2, b2):
    global LAST_EXEC_NS
    LAST_EXEC_NS = []
    x = np.asarray(x, np.float32)
    W1 = np.asarray(W1, np.float32)
    b1 = np.asarray(b1, np.float32)
    W2 = np.asarray(W2, np.float32)
    b2 = np.asarray(b2, np.float32)
    N, IN = x.shape
    HID = W1.shape[1]
    OUT = W2.shape[1]
    assert N % NCORES == 0
    SH = N // NCORES
    SHP = ((SH + 127) // 128) * 128
    R = SHP // 128
    trace = bool(os.environ.get("BASS_TRACE"))

    deg, degps, perms, K, spans, NSLOT, srcs_all = _preprocess(edge_index, N, SH, SHP)

    # ---- launch A
    key_a = ("A", SHP, IN, HID)
    if key_a not in _cache:
        _cache[key_a] = _build_launch_a(SHP, IN, HID)
    nc_a = _cache[key_a]
    in_maps = []
    for c in range(NCORES):
        xs = np.zeros((IN, SHP), np.float32)
        xs[:, :SH] = x[c * SH:(c + 1) * SH].T
        deg2d = np.ascontiguousarray(
            degps[c].reshape(128, R).astype(np.int32))
        in_maps.append({"xT": xs, "deg2d": deg2d, "w1": W1})
    res_a = run_bass_kernel_spmd(nc_a, in_maps, list(range(NCORES)), trace=trace)
    LAST_EXEC_NS.append(res_a.exec_time_ns)
    h1rows = np.empty((N + 1, HID), dtype=ml_dtypes.bfloat16)
    h1rows[N] = 0
    disP_all = []
    for c in range(NCORES):
        h1rows[c * SH:(c + 1) * SH] = res_a.results[c]["h1T"].T[:SH]  # [SH, HID]
        dis_nat = res_a.results[c]["dis2d"].reshape(-1)            # [SHP] p-major
        P2 = perms[c].reshape(R, 128)
        disP_all.append(np.ascontiguousarray(dis_nat[P2].T))       # [128, R]

    # ---- launch B
    chunks = _chunk_spans(spans, HID, 1536)
    key_b = ("B", SHP, HID, OUT, tuple(K.tolist()))
    if key_b not in _cache:
        _cache[key_b] = _build_launch_b(SHP, HID, OUT, spans, chunks, NSLOT)
    nc_b = _cache[key_b]
    b1rep = np.tile(b1, R)[None, :].repeat(128, axis=0).astype(np.float32)
    w2rep0 = np.tile(W2[:, 0], R)[None, :].repeat(128, axis=0).astype(np.float32)
    w2rep1 = np.tile(W2[:, 1], R)[None, :].repeat(128, axis=0).astype(np.float32)
    in_maps = []
    for c in range(NCORES):
        msg1 = _assemble_msg(h1rows, srcs_all[c], spans, HID)
        disrep = np.repeat(disP_all[c], HID, axis=1)
        in_maps.append({
            "msg1": msg1, "disrep": disrep, "b1rep": b1rep,
            "w2rep0": w2rep0, "w2rep1": w2rep1, "disP": disP_all[c],
        })
    res_b = run_bass_kernel_spmd(nc_b, in_maps, list(range(NCORES)), trace=trace)
    LAST_EXEC_NS.append(res_b.exec_time_ns)

    z2rows = np.empty((N + 1, OUT), dtype=ml_dtypes.bfloat16)
    z2rows[N] = 0
    for c in range(NCORES):
        P2 = perms[c].reshape(R, 128)
        pr = P2.ravel()
        msk = pr < SH
        z2rows[c * SH + pr[msk], 0] = res_b.results[c]["z0"].T.ravel()[msk]
        z2rows[c * SH + pr[msk], 1] = res_b.results[c]["z1"].T.ravel()[msk]

    # ---- launch C
    key_c = ("C", SHP, OUT, tuple(K.tolist()))
    if key_c not in _cache:
        _cache[key_c] = _build_launch_c(SHP, OUT, spans, NSLOT)
    nc_c = _cache[key_c]
    b2rep = np.tile(b2, R)[None, :].repeat(128, axis=0).astype(np.float32)
    in_maps = []
    for c in range(NCORES):
        msg2 = _assemble_msg(z2rows, srcs_all[c], spans, OUT)
        disrep2 = np.repeat(disP_all[c], OUT, axis=1)
        in_maps.append({"msg2": msg2, "disrep2": disrep2, "b2rep": b2rep})
    res_c = run_bass_kernel_spmd(nc_c, in_maps, list(range(NCORES)), trace=trace)
    LAST_EXEC_NS.append(res_c.exec_time_ns)

    out = np.empty((N, OUT), np.float32)
    for c in range(NCORES):
        P2 = perms[c].reshape(R, 128)
        pr = P2.ravel()
        msk = pr < SH
        out[c * SH + pr[msk], 0] = res_c.results[c]["o0"].T.ravel()[msk]
        out[c * SH + pr[msk], 1] = res_c.results[c]["o1"].T.ravel()[msk]
    return out


# revision 3
# speedup vs baseline: 1.0171x; 1.0010x over previous
"""2-layer GCN (GCNConv -> relu -> GCNConv -> log_softmax) on 8 NeuronCores.

Strategy (distributed GNN data parallel, dst-sharded):
  - nodes partitioned into 8 contiguous dst-shards; core c owns shard c
  - per-shard neighbor lists are degree-sorted and K-padded into uniform
    rank blocks of 128 dsts (shared K profile across cores => one NEFF)
  - launch A (per core): dis = rsqrt(deg); h1T = W1^T @ x^T scaled by
    dis[s], emitted as bf16 node rows (the layer-1 message table)
  - host exchanges halo rows: the per-edge message streams for each core
    are assembled by pure byte movement (np.take of device-produced bf16
    rows, zero-padding via a sentinel row) - no host float arithmetic
  - launch B (per core): stream msg1, segment-reduce over K slots on DVE,
    epilogue: *dis[d] +b1, relu, @W2 (via broadcast-multiply + reduce),
    *dis[d]  -> layer-2 message rows (bf16)
  - host exchanges halo rows again (msg2 assembly)
  - launch C (per core): stream msg2, segment-reduce, *dis[d] +b2,
    log_softmax -> output rows

All floating point math runs on device in fp32 (message streams are bf16).
Host does integer graph preprocessing and data movement only.
"""
import os
import sys

sys.path.insert(0, '/opt/trn_rl_repo')

import numpy as np
import ml_dtypes

from concourse import bass, bacc, mybir
import concourse.tile as tile
from concourse.bass_utils import run_bass_kernel_spmd

F32 = mybir.dt.float32
BF16 = mybir.dt.bfloat16
I32 = mybir.dt.int32

NCORES = 8

LAST_EXEC_NS = []

_cache = {}


# ----------------------------------------------------------------- launch A
def _build_launch_a(SHP, IN, HID):
    """Per core: dis2d = rsqrt(deg2d);  h1T[:, n] = dis[n] * (W1^T x[n])
    in bf16. Node id n = p*R + t (natural shard order; deg2d is the
    p-major reshape of the shard's deg)."""
    R = SHP // 128
    NSL = SHP // 512  # 512-column matmul slices
    nc = bacc.Bacc("TRN2", target_bir_lowering=False, debug=False, num_devices=NCORES)
    xT_d = nc.dram_tensor("xT", [IN, SHP], BF16, kind="ExternalInput")
    deg_d = nc.dram_tensor("deg2d", [128, R], I32, kind="ExternalInput")
    w1_d = nc.dram_tensor("w1", [IN, HID], BF16, kind="ExternalInput")
    h1T_d = nc.dram_tensor("h1T", [HID, SHP], BF16, kind="ExternalOutput")
    dis_d = nc.dram_tensor("dis2d", [128, R], F32, kind="ExternalOutput")

    with tile.TileContext(nc) as tc:
        with (
            tc.tile_pool(name="persist", bufs=1) as pp,
            tc.tile_pool(name="loop", bufs=3) as lp,
            tc.tile_pool(name="psum", bufs=4, space="PSUM") as psp,
            tc.tile_pool(name="dram", bufs=1, space="DRAM") as dp,
        ):
            w1 = pp.tile([IN, HID], BF16)
            nc.sync.dma_start(out=w1[:], in_=w1_d[:])
            degt = pp.tile([128, R], I32)
            nc.sync.dma_start(out=degt[:], in_=deg_d[:])
            degf = pp.tile([128, R], F32)
            nc.vector.tensor_copy(out=degf[:], in_=degt[:])
            dis = pp.tile([128, R], F32)
            nc.vector.reciprocal(out=dis[:], in_=degf[:])
            nc.scalar.activation(dis[:], dis[:], mybir.ActivationFunctionType.Sqrt)
            nc.sync.dma_start(out=dis_d[:], in_=dis[:])
            # roundtrip: [128, R] p-major -> DRAM [SH] -> replicated [HID, SH]
            disl = dp.tile([1, SHP], F32)
            nc.sync.dma_start(out=disl[:], in_=dis[:])
            dis16 = pp.tile([HID, SHP], F32)
            for h in range(HID):
                nc.sync.dma_start(out=dis16[h:h + 1, :], in_=disl[:])

            h1T = pp.tile([HID, SHP], BF16)
            for s in range(NSL):
                sl = slice(s * 512, (s + 1) * 512)
                xt = lp.tile([IN, 512], BF16, tag="xt")
                nc.sync.dma_start(out=xt[:], in_=xT_d[:, sl])
                ph = psp.tile([HID, 512], F32, tag="ph")
                nc.tensor.matmul(out=ph[:], lhsT=w1[:], rhs=xt[:],
                                 start=True, stop=True)
                nc.vector.tensor_tensor(out=h1T[:, sl], in0=ph[:],
                                        in1=dis16[:, sl], op=mybir.AluOpType.mult)
            nc.sync.dma_start(out=h1T_d[:], in_=h1T[:])
    nc.compile()
    return nc


# --------------------------------------------------------------- launch B
def _build_launch_b(SHP, HID, OUT, spans, chunks, NSLOT):
    """Stream msg1 [128, NSLOT*HID] bf16, reduce each span's K slots,
    epilogue -> z rows (bf16) for layer 2."""
    R = SHP // 128
    nc = bacc.Bacc("TRN2", target_bir_lowering=False, debug=False, num_devices=NCORES)
    msg_d = nc.dram_tensor("msg1", [128, NSLOT * HID], BF16, kind="ExternalInput")
    disrep_d = nc.dram_tensor("disrep", [128, R * HID], F32, kind="ExternalInput")
    b1rep_d = nc.dram_tensor("b1rep", [128, R * HID], F32, kind="ExternalInput")
    w2rep0_d = nc.dram_tensor("w2rep0", [128, R * HID], F32, kind="ExternalInput")
    w2rep1_d = nc.dram_tensor("w2rep1", [128, R * HID], F32, kind="ExternalInput")
    disP_d = nc.dram_tensor("disP", [128, R], F32, kind="ExternalInput")
    z0_d = nc.dram_tensor("z0", [128, R], BF16, kind="ExternalOutput")
    z1_d = nc.dram_tensor("z1", [128, R], BF16, kind="ExternalOutput")

    with tile.TileContext(nc) as tc:
        with (
            tc.tile_pool(name="persist", bufs=1) as pp,
            tc.tile_pool(name="msg", bufs=2) as mp,
        ):
            disrep = pp.tile([128, R * HID], F32)
            nc.sync.dma_start(out=disrep[:], in_=disrep_d[:])
            b1rep = pp.tile([128, R * HID], F32)
            nc.sync.dma_start(out=b1rep[:], in_=b1rep_d[:])
            agg = pp.tile([128, R * HID], F32)
            for (s0, s1, r0a, spl) in chunks:
                nslots = s1 - s0
                m = mp.tile([128, nslots * HID], BF16, tag="m")
                nc.sync.dma_start(out=m[:], in_=msg_d[:, s0 * HID:s1 * HID])
                for (K, r0, r1, soff) in spl:
                    nr = r1 - r0
                    view = m[:, (soff - s0) * HID:(soff - s0 + nr * K) * HID] \
                        .rearrange('p (nr f k) -> p nr f k', f=HID, k=K)
                    nc.vector.tensor_reduce(
                        out=agg[:, r0 * HID:r1 * HID], in_=view,
                        axis=mybir.AxisListType.X, op=mybir.AluOpType.add)
            # epilogue: h = relu(agg*disrep + b1)
            nc.vector.tensor_tensor(out=agg[:], in0=agg[:], in1=disrep[:],
                                    op=mybir.AluOpType.mult)
            nc.vector.tensor_tensor(out=agg[:], in0=agg[:], in1=b1rep[:],
                                    op=mybir.AluOpType.add)
            nc.vector.tensor_scalar_max(agg[:], agg[:], 0.0)
            # z_o = (sum_f h*w2rep_o) * disP  -> bf16
            disP = pp.tile([128, R], F32)
            nc.sync.dma_start(out=disP[:], in_=disP_d[:])
            tmp = pp.tile([128, R * HID], F32)
            for o, (w_d, z_d) in enumerate(((w2rep0_d, z0_d), (w2rep1_d, z1_d))):
                w2rep = mp.tile([128, R * HID], F32, tag="w2rep")
                nc.sync.dma_start(out=w2rep[:], in_=w_d[:])
                nc.vector.tensor_tensor(out=tmp[:], in0=agg[:], in1=w2rep[:],
                                        op=mybir.AluOpType.mult)
                z = pp.tile([128, R], F32)
                nc.vector.tensor_reduce(
                    out=z[:], in_=tmp[:].rearrange('p (r f) -> p r f', f=HID),
                    axis=mybir.AxisListType.X, op=mybir.AluOpType.add)
                nc.vector.tensor_tensor(out=z[:], in0=z[:], in1=disP[:],
                                        op=mybir.AluOpType.mult)
                zb = pp.tile([128, R], BF16)
                nc.vector.tensor_copy(out=zb[:], in_=z[:])
                nc.sync.dma_start(out=z_d[:], in_=zb[:])
    nc.compile()
    return nc


# --------------------------------------------------------------- launch C
def _build_launch_c(SHP, OUT, spans, NSLOT):
    """Stream msg2 [128, NSLOT*OUT] bf16, reduce, *dis +b2, log_softmax."""
    R = SHP // 128
    nc = bacc.Bacc("TRN2", target_bir_lowering=False, debug=False, num_devices=NCORES)
    msg_d = nc.dram_tensor("msg2", [128, NSLOT * OUT], BF16, kind="ExternalInput")
    disrep_d = nc.dram_tensor("disrep2", [128, R * OUT], F32, kind="ExternalInput")
    b2rep_d = nc.dram_tensor("b2rep", [128, R * OUT], F32, kind="ExternalInput")
    o0_d = nc.dram_tensor("o0", [128, R], F32, kind="ExternalOutput")
    o1_d = nc.dram_tensor("o1", [128, R], F32, kind="ExternalOutput")

    with tile.TileContext(nc) as tc:
        with (
            tc.tile_pool(name="persist", bufs=1) as pp,
            tc.tile_pool(name="msg", bufs=2) as mp,
        ):
            agg = pp.tile([128, R * OUT], F32)
            half = NSLOT // 2
            # split spans at ~half for double buffering
            bnds = [0]
            acc = 0
            spl_chunks = [[]]
            for (K, r0, r1, soff) in spans:
                if acc >= half and len(spl_chunks) == 1:
                    spl_chunks.append([])
                    bnds.append(acc)
                spl_chunks[-1].append((K, r0, r1, soff))
                acc = soff + (r1 - r0) * K
            bnds.append(NSLOT)
            for ci, spl in enumerate(spl_chunks):
                s0, s1 = bnds[ci], bnds[ci + 1]
                if s1 <= s0:
                    continue
                m = mp.tile([128, (s1 - s0) * OUT], BF16, tag="m")
                nc.sync.dma_start(out=m[:], in_=msg_d[:, s0 * OUT:s1 * OUT])
                for (K, r0, r1, soff) in spl:
                    nr = r1 - r0
                    view = m[:, (soff - s0) * OUT:(soff - s0 + nr * K) * OUT] \
                        .rearrange('p (nr f k) -> p nr f k', f=OUT, k=K)
                    nc.vector.tensor_reduce(
                        out=agg[:, r0 * OUT:r1 * OUT], in_=view,
                        axis=mybir.AxisListType.X, op=mybir.AluOpType.add)
            disrep = pp.tile([128, R * OUT], F32)
            nc.sync.dma_start(out=disrep[:], in_=disrep_d[:])
            b2rep = pp.tile([128, R * OUT], F32)
            nc.sync.dma_start(out=b2rep[:], in_=b2rep_d[:])
            nc.vector.tensor_tensor(out=agg[:], in0=agg[:], in1=disrep[:],
                                    op=mybir.AluOpType.mult)
            nc.vector.tensor_tensor(out=agg[:], in0=agg[:], in1=b2rep[:],
                                    op=mybir.AluOpType.add)
            # log_softmax over the 2 classes
            a3 = agg[:].rearrange('p (r o) -> p r o', o=OUT)
            z0v, z1v = a3[:, :, 0:1], a3[:, :, 1:2]
            mx = pp.tile([128, R], F32)
            nc.vector.tensor_tensor(out=mx[:], in0=z0v, in1=z1v, op=mybir.AluOpType.max)
            d0 = pp.tile([128, R], F32)
            d1 = pp.tile([128, R], F32)
            nc.vector.tensor_tensor(out=d0[:], in0=z0v, in1=mx[:], op=mybir.AluOpType.subtract)
            nc.vector.tensor_tensor(out=d1[:], in0=z1v, in1=mx[:], op=mybir.AluOpType.subtract)
            e0 = pp.tile([128, R], F32)
            e1 = pp.tile([128, R], F32)
            nc.scalar.activation(e0[:], d0[:], mybir.ActivationFunctionType.Exp)
            nc.scalar.activation(e1[:], d1[:], mybir.ActivationFunctionType.Exp)
            nc.vector.tensor_tensor(out=e0[:], in0=e0[:], in1=e1[:], op=mybir.AluOpType.add)
            ls = pp.tile([128, R], F32)
            nc.scalar.activation(ls[:], e0[:], mybir.ActivationFunctionType.Ln)
            nc.vector.tensor_tensor(out=d0[:], in0=d0[:], in1=ls[:], op=mybir.AluOpType.subtract)
            nc.vector.tensor_tensor(out=d1[:], in0=d1[:], in1=ls[:], op=mybir.AluOpType.subtract)
            nc.sync.dma_start(out=o0_d[:], in_=d0[:])
            nc.sync.dma_start(out=o1_d[:], in_=d1[:])
    nc.compile()
    return nc


# ------------------------------------------------------------- preprocessing
def _preprocess(edge_index, N, SH, SHP):
    """Degree-sorted K-padded per-core layouts (shared K profile).
    Shards padded from SH to SHP dsts (padded dsts: one sentinel slot)."""
    src = np.asarray(edge_index[0]).astype(np.int64)
    dst = np.asarray(edge_index[1]).astype(np.int64)
    deg = (np.bincount(dst, minlength=N) + 1).astype(np.int64)  # incl. self
    R = SHP // 128

    perms = []
    csrs = []   # (order-sorted srcs per core, start offsets per local dst)
    degps = []
    for c in range(NCORES):
        lo, hi = c * SH, (c + 1) * SH
        degp = np.ones(SHP, np.int64)
        degp[:SH] = deg[lo:hi]
        degps.append(degp)
        perm = np.argsort(-degp, kind='stable')   # local ids, degree desc
        perms.append(perm)
        sel = (dst >= lo) & (dst < hi)
        ds = dst[sel] - lo
        ss = src[sel]
        order = np.argsort(ds, kind='stable')
        ss = ss[order]
        counts = np.bincount(ds, minlength=SHP)
        starts = np.zeros(SHP + 1, np.int64)
        np.cumsum(counts, out=starts[1:])
        csrs.append((ss, starts))

    # shared K profile: K[r] = max over cores of the rank's max degree
    K = np.zeros(R, np.int64)
    for c in range(NCORES):
        sd = degps[c][perms[c]]
        K = np.maximum(K, sd[0::128])
    K = np.maximum(K, 1)

    # spans of equal K: (K, r0, r1, slot_offset)
    spans = []
    r0 = 0
    soff = 0
    for r in range(1, R + 1):
        if r == R or K[r] != K[r0]:
            spans.append((int(K[r0]), r0, r, soff))
            soff += (r - r0) * int(K[r0])
            r0 = r
    NSLOT = int(K.sum())

    # srcs_pad per core: [128, NSLOT] int32 global ids; sentinel = N
    srcs_all = np.empty((NCORES, 128, NSLOT), np.int32)
    for c in range(NCORES):
        perm = perms[c]
        P2 = perm.reshape(R, 128)                  # P2[r, p] = local dst id
        ss, starts = csrs[c]
        ss_ext = np.concatenate([ss, [np.int64(N)]])
        for (Kr, r0, r1, soff) in spans:
            for r in range(r0, r1):
                dl = P2[r]                          # [128] local ids
                dg = np.where(dl < SH, dl + c * SH, N)  # padded -> sentinel
                blk = np.full((128, Kr), N, np.int64)
                blk[:, 0] = dg                      # self-loop
                if Kr > 1:
                    lens = starts[dl + 1] - starts[dl]
                    ti = starts[dl][:, None] + np.arange(Kr - 1)[None, :]
                    valid = np.arange(Kr - 1)[None, :] < lens[:, None]
                    ti = np.where(valid, ti, len(ss))
                    blk[:, 1:] = ss_ext[ti]
                off = soff + (r - r0) * Kr
                srcs_all[c, :, off:off + Kr] = blk.astype(np.int32)

    return deg, degps, perms, K, spans, NSLOT, srcs_all


def _chunk_spans(spans, HID, budget_slots):
    """Split the span list into DMA chunks of <= budget_slots slots;
    large spans are split by ranks. Returns list of
    (s0, s1, r0, [(K, r0, r1, soff), ...])."""
    # first split big spans by rank
    fine = []
    for (K, r0, r1, soff) in spans:
        nr = r1 - r0
        max_nr = max(1, budget_slots // K)
        rr = r0
        so = soff
        while rr < r1:
            take = min(max_nr, r1 - rr)
            fine.append((K, rr, rr + take, so))
            so += take * K
            rr += take
    chunks = []
    cur = []
    s0 = 0
    acc = 0
    for sp in fine:
        K, r0, r1, soff = sp
        sz = (r1 - r0) * K
        if cur and acc + sz > budget_slots:
            chunks.append((s0, s0 + acc, cur[0][1], cur))
            s0 += acc
            cur = []
            acc = 0
        cur.append(sp)
        acc += sz
    if cur:
        chunks.append((s0, s0 + acc, cur[0][1], cur))
    return chunks


def _assemble_msg(table_ext, srcs, spans, F):
    """table_ext [N+1, F] bf16, srcs [128, NSLOT] int32 ->
    msg [128, NSLOT*F] bf16 with per-span layout (p, nr, F, K)."""
    NSLOT = srcs.shape[1]
    msg = np.empty((128, NSLOT * F), dtype=ml_dtypes.bfloat16)
    for (K, r0, r1, soff) in spans:
        nr = r1 - r0
        blk = table_ext[srcs[:, soff:soff + nr * K].reshape(128, nr, K)]
        # [128, nr, K, F] -> [128, nr, F, K]
        blk = np.ascontiguousarray(blk.transpose(0, 1, 3, 2))
        msg[:, soff * F:(soff + nr * K) * F] = blk.reshape(128, nr * K * F)
    return msg


# ---------------------------------------------------------------------- main
def kernel(x, edge_index, W1, b1, W2, b2):
    global LAST_EXEC_NS
    LAST_EXEC_NS = []
    x = np.asarray(x, np.float32)
    W1 = np.asarray(W1, np.float32)
    b1 = np.asarray(b1, np.float32)
    W2 = np.asarray(W2, np.float32)
    b2 = np.asarray(b2, np.float32)
    N, IN = x.shape
    HID = W1.shape[1]
    OUT = W2.shape[1]
    assert N % NCORES == 0
    SH = N // NCORES
    SHP = ((SH + 127) // 128) * 128
    R = SHP // 128
    trace = bool(os.environ.get("BASS_TRACE"))

    deg, degps, perms, K, spans, NSLOT, srcs_all = _preprocess(edge_index, N, SH, SHP)

    # ---- launch A
    key_a = ("A", SHP, IN, HID)
    if key_a not in _cache:
        _cache[key_a] = _build_launch_a(SHP, IN, HID)
    nc_a = _cache[key_a]
    in_maps = []
    for c in range(NCORES):
        xs = np.zeros((IN, SHP), ml_dtypes.bfloat16)
        xs[:, :SH] = x[c * SH:(c + 1) * SH].T.astype(ml_dtypes.bfloat16)
        deg2d = np.ascontiguousarray(
            degps[c].reshape(128, R).astype(np.int32))
        in_maps.append({"xT": xs, "deg2d": deg2d,
                        "w1": W1.astype(ml_dtypes.bfloat16)})
    res_a = run_bass_kernel_spmd(nc_a, in_maps, list(range(NCORES)), trace=trace)
    LAST_EXEC_NS.append(res_a.exec_time_ns)
    h1rows = np.empty((N + 1, HID), dtype=ml_dtypes.bfloat16)
    h1rows[N] = 0
    disP_all = []
    for c in range(NCORES):
        h1rows[c * SH:(c + 1) * SH] = res_a.results[c]["h1T"].T[:SH]  # [SH, HID]
        dis_nat = res_a.results[c]["dis2d"].reshape(-1)            # [SHP] p-major
        P2 = perms[c].reshape(R, 128)
        disP_all.append(np.ascontiguousarray(dis_nat[P2].T))       # [128, R]

    # ---- launch B
    chunks = _chunk_spans(spans, HID, 1536)
    key_b = ("B", SHP, HID, OUT, tuple(K.tolist()))
    if key_b not in _cache:
        _cache[key_b] = _build_launch_b(SHP, HID, OUT, spans, chunks, NSLOT)
    nc_b = _cache[key_b]
    b1rep = np.tile(b1, R)[None, :].repeat(128, axis=0).astype(np.float32)
    w2rep0 = np.tile(W2[:, 0], R)[None, :].repeat(128, axis=0).astype(np.float32)
    w2rep1 = np.tile(W2[:, 1], R)[None, :].repeat(128, axis=0).astype(np.float32)
    in_maps = []
    for c in range(NCORES):
        msg1 = _assemble_msg(h1rows, srcs_all[c], spans, HID)
        disrep = np.repeat(disP_all[c], HID, axis=1)
        in_maps.append({
            "msg1": msg1, "disrep": disrep, "b1rep": b1rep,
            "w2rep0": w2rep0, "w2rep1": w2rep1, "disP": disP_all[c],
        })
    res_b = run_bass_kernel_spmd(nc_b, in_maps, list(range(NCORES)), trace=trace)
    LAST_EXEC_NS.append(res_b.exec_time_ns)

    z2rows = np.empty((N + 1, OUT), dtype=ml_dtypes.bfloat16)
    z2rows[N] = 0
    for c in range(NCORES):
        P2 = perms[c].reshape(R, 128)
        pr = P2.ravel()
        msk = pr < SH
        z2rows[c * SH + pr[msk], 0] = res_b.results[c]["z0"].T.ravel()[msk]
        z2rows[c * SH + pr[msk], 1] = res_b.results[c]["z1"].T.ravel()[msk]

    # ---- launch C
    key_c = ("C", SHP, OUT, tuple(K.tolist()))
    if key_c not in _cache:
        _cache[key_c] = _build_launch_c(SHP, OUT, spans, NSLOT)
    nc_c = _cache[key_c]
    b2rep = np.tile(b2, R)[None, :].repeat(128, axis=0).astype(np.float32)
    in_maps = []
    for c in range(NCORES):
        msg2 = _assemble_msg(z2rows, srcs_all[c], spans, OUT)
        disrep2 = np.repeat(disP_all[c], OUT, axis=1)
        in_maps.append({"msg2": msg2, "disrep2": disrep2, "b2rep": b2rep})
    res_c = run_bass_kernel_spmd(nc_c, in_maps, list(range(NCORES)), trace=trace)
    LAST_EXEC_NS.append(res_c.exec_time_ns)

    out = np.empty((N, OUT), np.float32)
    for c in range(NCORES):
        P2 = perms[c].reshape(R, 128)
        pr = P2.ravel()
        msk = pr < SH
        out[c * SH + pr[msk], 0] = res_c.results[c]["o0"].T.ravel()[msk]
        out[c * SH + pr[msk], 1] = res_c.results[c]["o1"].T.ravel()[msk]
    return out
